# revision 47
# baseline (speedup 1.0000x reference)
"""Trainium2 Bass kernel for nn_Decoder (latent MLP -> GRU scan -> per-step MLP).

Strategy: pure data-parallel over batch (4096 -> 8 x 512), weights replicated.
On-chip layout is transposed (features on partitions, batch on free dim).

v2 restructure vs v1 (same math; fewer/bigger ops, shorter serial cycle):
 - Element-wise gate work is batched into half-gate [128, 2, 512]
   instructions spanning two PSUM banks (one ACT sigmoid / one DVE op per
   two feature tiles), roughly halving ACT/DVE instruction count.
 - All biases are folded into matmul accumulation: x carries a constant
   ones row so W_ih-side biases ride the gi matmul (x and W_ih are fp8
   k-pair packed, DoubleRow); h-side biases (b_hh n-part, bm1) are fp8
   DR bias-row matmuls in the same PSUM group.
 - The n-gate add s = inn + r*hn costs only ONE DVE op (rhn): the +inn
   lands via an identity matmul accumulated into inn's open PSUM group on
   the otherwise-idle PE, and tanh(n) reads PSUM directly.
 - The GRU blend is h' = (n - z*n) + w with w = z*h computed on Pool OFF
   the serial chain (z and old h are ready early); the on-chain tail
   after tanh(n) is three short DVE ops per half.
 - Recurrent matmuls (W_hh, Wm1, Wm2) are fp8e4m3 DoubleRow; hidden state
   h and pred intermediate p1 live in fp8 pair-layout tiles [128, 4, 512].
 - Pred for step t-1 is emitted after the tail of step t; outputs staged
   8 steps at a time and DMAd per batch-tile.
 - TimelineSim (cost model) predicts ~11.3 us/step vs the baseline's
   12.25; hardware tracks the cost model at a uniform ~1.63x.

Self-contained: hardcodes shapes from the problem spec.
"""
import sys
sys.path.insert(0, "/opt/trn_rl_repo")
from contextlib import ExitStack

import numpy as np
import ml_dtypes

import concourse.bacc as bacc
import concourse.mybir as mybir
from concourse import tile
from concourse import bass_utils

BF16 = ml_dtypes.bfloat16
FP8 = ml_dtypes.float8_e4m3
BF = mybir.dt.bfloat16
F8 = mybir.dt.float8e4
F32 = mybir.dt.float32
AF = mybir.ActivationFunctionType
ALU = mybir.AluOpType
DR = mybir.MatmulPerfMode.DoubleRow

N_CORES = 8
B, LAT, H, A, L = 4096, 256, 512, 64, 128
BOS = 0
T = L - 1          # recurrence steps
BL = B // N_CORES  # per-core batch
KH = H // 128      # feature tiles per H (4)
AX = A + 1         # x rows incl the constant ones row


def _build(steps=T, n_cores=N_CORES, reps=1, timing_iters=None, unroll=8,
           variant="full"):
    """Always declares the full-size DRAM interface (xT[T], y[:, L]); `steps`
    bounds the recurrence so short builds are wall-clock comparable.

    timing_iters: if set, wraps `unroll` statically-addressed step bodies in a
    hardware For_i loop executed timing_iters//unroll times (numerics garbage,
    per-step work identical) — used only to measure per-step device time."""
    nc = bacc.Bacc("TRN2", target_bir_lowering=False, debug=False,
                   num_devices=n_cores)

    d = {}
    def din(name, shape, dt=BF):
        d[name] = nc.dram_tensor(name, list(shape), dt, kind="ExternalInput").ap()

    din("latentT", [LAT, BL])
    din("xT", [T, 33, 2, BL], F8)         # x k-pair packed (rows 64=ones, 65=0)
    din("WhhD", [2, 128, 2, 3 * H], F8)   # k-pair-packed W_hh.T (z negated)
    din("Wm1D", [2, 128, 2, H], F8)
    din("Wm2D", [2, 128, 2, A], F8)
    din("WihA", [33, 2, 3 * H], F8)       # W_ih.T + bias row, k-pair packed
    din("Wm3T", [A, A])
    din("Wd1T", [LAT, H])
    din("Wd2T", [H, H])
    din("Wd3T", [H, H])
    din("bhnn_r", [1, 2, H], F8)          # b_hh n-part as a DR pair row
    din("bm1_r", [1, 2, H], F8)
    din("ident", [128, 128])              # bf16 identity for PE adds
    din("bm2", [A], F32)
    din("bm3b4", [128, 4 * A], F32)
    din("bd1", [H], F32)
    din("bd2", [H], F32)
    din("bd3", [H], F32)
    y = nc.dram_tensor("y", [BL, L, A], F32, kind="ExternalOutput").ap()

    with tile.TileContext(nc) as tc, ExitStack() as ctx:
        cst = ctx.enter_context(tc.tile_pool(name="const", bufs=1))
        wrk = ctx.enter_context(tc.tile_pool(name="work", bufs=2))
        hpool = ctx.enter_context(tc.tile_pool(name="hp", bufs=3))
        ps = ctx.enter_context(tc.tile_pool(name="ps", bufs=4, space="PSUM"))

        def const_tile(shape, dt, tag, src):
            t = cst.tile(list(shape), dt, tag=tag, name=tag)
            nc.sync.dma_start(t[:], src)
            return t

        whhd = [const_tile([128, 2, 3 * H], F8, f"whhd{p}", d["WhhD"][p])
                for p in range(2)]
        wm1d = [const_tile([128, 2, H], F8, f"wm1d{p}", d["Wm1D"][p])
                for p in range(2)]
        wm2d = [const_tile([128, 2, A], F8, f"wm2d{p}", d["Wm2D"][p])
                for p in range(2)]
        wih = const_tile([33, 2, 3 * H], F8, "wih", d["WihA"][:])
        wm3 = const_tile([A, A], BF, "wm3", d["Wm3T"][:])
        wd1 = [const_tile([128, H], BF, f"wd1{k}",
                          d["Wd1T"][k * 128:(k + 1) * 128, :]) for k in range(2)]
        wd2 = [const_tile([128, H], BF, f"wd2{k}",
                          d["Wd2T"][k * 128:(k + 1) * 128, :]) for k in range(KH)]
        wd3 = [const_tile([128, H], BF, f"wd3{k}",
                          d["Wd3T"][k * 128:(k + 1) * 128, :]) for k in range(KH)]
        bhnn = const_tile([1, 2, H], F8, "bhnn", d["bhnn_r"][:])
        bm1r = const_tile([1, 2, H], F8, "bm1r", d["bm1_r"][:])
        ident = const_tile([128, 128], BF, "ident", d["ident"][:])
        bm2 = const_tile([A, 1], F32, "bm2", d["bm2"][:, None])
        bm3b4 = const_tile([128, 4 * A], F32, "bm3b4", d["bm3b4"][:])

        def bias_tiles(name, n, tag):
            return [const_tile([128, 1], F32, f"{tag}{j}",
                               d[name][j * 128:(j + 1) * 128, None])
                    for j in range(n)]

        bd1 = bias_tiles("bd1", KH, "bd1")
        bd2 = bias_tiles("bd2", KH, "bd2")
        bd3 = bias_tiles("bd3", KH, "bd3")

        ones = cst.tile([1, 2, BL], F8, tag="ones", name="ones")
        nc.vector.memset(ones[:], 1.0)

        lat = [const_tile([128, BL], BF, f"lat{k}",
                          d["latentT"][k * 128:(k + 1) * 128, :]) for k in range(2)]

        # ---- init MLP: latent -> h0, final layer written into fp8 pairs
        def mlp_layer(w_tiles, rhs_tiles, bias, act, out_tag):
            outs = []
            for m in range(KH):
                acc = ps.tile([128, 2, 512], F32, tag="g", name="ps")
                nk = len(rhs_tiles)
                for k in range(nk):
                    nc.tensor.matmul(
                        acc[:, 0, :], w_tiles[k][:, m * 128:(m + 1) * 128],
                        rhs_tiles[k][:], start=(k == 0), stop=(k == nk - 1))
                o = wrk.tile([128, BL], BF, tag=f"{out_tag}{m}",
                             name=f"{out_tag}{m}")
                nc.scalar.activation(o[:], acc[:, 0, :], act, bias=bias[m][:])
                outs.append(o)
            return outs

        h1 = mlp_layer(wd1, lat, bd1, AF.Tanh, "h1")
        h2 = mlp_layer(wd2, h1, bd2, AF.Tanh, "h2")
        h8_init = hpool.tile([128, KH, BL], F8, tag="h8", name="h8")
        for m in range(KH):
            acc = ps.tile([128, 2, 512], F32, tag="g", name="ps")
            for k in range(KH):
                nc.tensor.matmul(
                    acc[:, 0, :], wd3[k][:, m * 128:(m + 1) * 128],
                    h2[k][:], start=(k == 0), stop=(k == KH - 1))
            nc.scalar.activation(h8_init[:, m, :], acc[:, 0, :], AF.Identity,
                                 bias=bd3[m][:])

        # hist[t] = fp8 h tile of step t (init state = hist[-1]); pred for
        # step t is emitted after the tail of step t+1
        state = {"hist": {-1: h8_init}, "ystage": None, "xts": {}}

        def fetch_x(t):
            if t in state["xts"]:
                return
            xt = wrk.tile([33, 2, BL], F8, tag="xt", name="xt", bufs=4)
            nc.sync.dma_start(xt[:], d["xT"][t])
            state["xts"][t] = xt

        def gate_half(hh, xt, h8, base, wdr, bias_row=None):
            """PSUM half-gate [128,2,512] for feature tiles jj=2hh+m2 of a
            gate; wdr columns are sliced at base+jj*128 (base in
            {0,H,2H} for whhd, 0 for wm1d); gi matmul iff xt given."""
            acc = ps.tile([128, 2, 512], F32, tag="g", name="ps")
            for m2 in range(2):
                jj = 2 * hh + m2
                c0 = base + jj * 128
                o = acc[:, m2, :]
                first = True
                if xt is not None:
                    nc.tensor.matmul(o, wih[:, :, c0:c0 + 128], xt[:],
                                     start=True, stop=False, perf_mode=DR)
                    first = False
                for p in range(2):
                    nc.tensor.matmul(
                        o, wdr[p][:, :, c0:c0 + 128],
                        h8[:, 2 * p:2 * p + 2, :],
                        start=(first and p == 0),
                        stop=(p == 1 and bias_row is None),
                        perf_mode=DR)
                if bias_row is not None:
                    nc.tensor.matmul(
                        o, bias_row[:, :, jj * 128:(jj + 1) * 128],
                        ones[:], start=False, stop=True, perf_mode=DR)
            return acc

        def emit_step(t, t_next):
            h8 = state["hist"][t - 1]
            fetch_x(t)
            xt = state["xts"].pop(t)
            if t_next is not None:
                fetch_x(t_next)   # prefetch next step's x under this step

            # PSUM "g" alloc order (bufs=4): rp0,zp0,rp1,zp1,hn0,hn1,
            # inn0,inn1 then pred's w10,w11,p2ps,yp. ACT order
            # r0,z0,r1,z1,n_h0,n_h1; z's early feed the off-chain Pool
            # w = z*h; the DVE tail runs as half-blocks.
            z_all = wrk.tile([128, KH, BL], BF, tag="z", name="z")
            r_all = wrk.tile([128, KH, BL], BF, tag="r", name="r")
            rp0 = gate_half(0, xt, h8, 0, whhd)
            nc.scalar.activation(r_all[:, 0:2, :], rp0[:], AF.Sigmoid)
            zp0 = gate_half(0, xt, h8, H, whhd)
            nc.scalar.activation(z_all[:, 0:2, :], zp0[:], AF.Sigmoid)
            rp1 = gate_half(1, xt, h8, 0, whhd)
            nc.scalar.activation(r_all[:, 2:4, :], rp1[:], AF.Sigmoid)
            zp1 = gate_half(1, xt, h8, H, whhd)
            nc.scalar.activation(z_all[:, 2:4, :], zp1[:], AF.Sigmoid)
            hnps = [gate_half(hh, None, h8, 2 * H, whhd, bias_row=bhnn)
                    for hh in range(2)]
            inns = []
            for hh in range(2):
                acc = ps.tile([128, 2, 512], F32, tag="g", name="ps")
                for m2 in range(2):
                    j = 8 + 2 * hh + m2
                    nc.tensor.matmul(acc[:, m2, :],
                                     wih[:, :, j * 128:(j + 1) * 128], xt[:],
                                     start=True, stop=False, perf_mode=DR)
                inns.append(acc)

            # w = z*h on Pool: off the serial chain
            ws = []
            for hh in range(2):
                w = wrk.tile([128, 2, BL], BF, tag=f"w{hh}", name=f"w{hh}")
                nc.gpsimd.tensor_mul(w[:], z_all[:, 2 * hh:2 * hh + 2, :],
                                     h8[:, 2 * hh:2 * hh + 2, :])
                ws.append(w)

            # n-gate pre-activation in PSUM: s = inn + r*hn; the +inn add
            # rides the PE as an identity matmul into inn's open group
            rhn = wrk.tile([128, KH, BL], BF, tag="rhn", name="rhn")
            for hh in range(2):
                sl = slice(2 * hh, 2 * hh + 2)
                nc.vector.tensor_mul(rhn[:, sl, :], hnps[hh][:],
                                     r_all[:, sl, :])
                for m2 in range(2):
                    nc.tensor.matmul(inns[hh][:, m2, :], ident[:],
                                     rhn[:, 2 * hh + m2, :],
                                     start=False, stop=(m2 == 1))

            # n + blend per half: h' = (n - z*n) + w
            n_all = wrk.tile([128, KH, BL], BF, tag="n", name="n")
            zn = wrk.tile([128, KH, BL], BF, tag="zn", name="zn")
            t2 = wrk.tile([128, KH, BL], BF, tag="t2", name="t2")
            h8_new = hpool.tile([128, KH, BL], F8, tag="h8", name="h8")
            for hh in range(2):
                sl = slice(2 * hh, 2 * hh + 2)
                nc.scalar.activation(n_all[:, sl, :], inns[hh][:], AF.Tanh)
                nc.vector.tensor_mul(zn[:, sl, :], z_all[:, sl, :],
                                     n_all[:, sl, :])
                nc.vector.tensor_sub(t2[:, sl, :], n_all[:, sl, :],
                                     zn[:, sl, :])
                nc.vector.tensor_add(h8_new[:, sl, :], t2[:, sl, :],
                                     ws[hh][:])
            state["hist"][t] = h8_new

        def emit_pred(tp, last, force=False):
            if tp < 0 and not force:
                return
            h8 = (state["hist"][tp] if tp in state["hist"]
                  else state["hist"][-1])
            ystage = state["ystage"]
            w1ps = [gate_half(hh, None, h8, 0, wm1d, bias_row=bm1r)
                    for hh in range(2)]
            p1f8 = wrk.tile([128, KH, BL], F8, tag="p1", name="p1")
            for hh in range(2):
                nc.scalar.activation(p1f8[:, 2 * hh:2 * hh + 2, :],
                                     w1ps[hh][:], AF.Tanh)
            p2ps = ps.tile([128, 2, 512], F32, tag="g", name="ps")
            for p in range(2):
                nc.tensor.matmul(p2ps[0:A, 0, :], wm2d[p][:, :, :],
                                 p1f8[:, 2 * p:2 * p + 2, :],
                                 start=(p == 0), stop=(p == 1), perf_mode=DR)
            p2 = wrk.tile([A, BL], BF, tag="p2", name="p2")
            nc.scalar.activation(p2[:], p2ps[0:A, 0, :], AF.Tanh,
                                 bias=bm2[:])

            tps = tp if tp >= 0 else tp + 8  # timing-build pseudo-slot
            o = (tps + 1) % 8
            g = (tps + 1) // 8
            if ystage is None or o == 0 or (g == 0 and o == 1):
                ystage = wrk.tile([128, 8, 4 * A], F32, tag="yst",
                                  name="yst")
            yp = ps.tile([128, 2, 512], F32, tag="g", name="ps")
            for bt in range(4):
                nc.tensor.matmul(yp[:, 0, bt * A:(bt + 1) * A],
                                 p2[:, bt * 128:(bt + 1) * 128],
                                 wm3[:], start=True, stop=True)
            nc.vector.tensor_add(ystage[:, o, :], yp[:, 0, 0:4 * A],
                                 bm3b4[:])
            if o == 7 or last:
                lo = 1 if g == 0 else 0
                hi = o + 1
                for bt in range(4):
                    nc.sync.dma_start(
                        y[bt * 128:(bt + 1) * 128, g * 8 + lo:g * 8 + hi, :],
                        ystage[:, lo:hi, bt * A:(bt + 1) * A])
            state["ystage"] = ystage
            state["hist"].pop(tp - 1, None)

        PRED_LAG = 1
        if timing_iters is None:
            for _rep in range(reps):
                for t in range(steps):
                    emit_step(t, t + 1 if t + 1 < steps else None)
                    emit_pred(t - PRED_LAG, last=False)
                for tp in range(max(steps - PRED_LAG, 0), steps):
                    emit_pred(tp, last=(tp == steps - 1))
        else:
            # timing loop: same per-step work (preds for t<LAG read init h —
            # numerics are garbage in timing builds anyway)
            with tc.For_i(0, timing_iters // unroll, 1):
                for t in range(unroll):
                    emit_step(t, (t + 1) % min(unroll, steps))
                    emit_pred(t - PRED_LAG, last=False, force=True)

    nc.compile()
    return nc


def _make_bos():
    bos = np.full((B, A), -16.0, np.float32)
    bos[:, BOS] = 16.0
    return bos


def _packd(WT):
    """[K, M] k-major -> [K//256, 128, 2, M] fp8 DoubleRow pair layout."""
    K, M = WT.shape
    return np.ascontiguousarray(
        WT.reshape(K // 256, 2, 128, M).transpose(0, 2, 1, 3)).astype(FP8)


def _make_in_maps(inputs, n_cores=N_CORES, T=T):
    bl = B // n_cores
    f32 = np.float32
    WhhT = np.ascontiguousarray(np.asarray(inputs["W_hh"], f32).T)
    Wm1T = np.ascontiguousarray(np.asarray(inputs["Wm1"], f32).T)
    Wm2T = np.ascontiguousarray(np.asarray(inputs["Wm2"], f32).T)
    b_ih = np.asarray(inputs["b_ih"], f32)
    b_hh = np.asarray(inputs["b_hh"], f32)
    # W_ih.T augmented with a bias row (r/z cols get b_ih+b_hh, n cols
    # b_ih) and a zero row, then packed into fp8 k-pairs [33, 2, 3H]
    brow = b_ih.copy()
    brow[:2 * H] += b_hh[:2 * H]
    WihA = np.concatenate(
        [np.asarray(inputs["W_ih"], f32).T, brow[None, :],
         np.zeros((1, 3 * H), f32)], axis=0).reshape(33, 2, 3 * H)
    shared = {
        "WhhD": _packd(WhhT),
        "Wm1D": _packd(Wm1T),
        "Wm2D": _packd(Wm2T),
        "WihA": np.ascontiguousarray(WihA).astype(FP8),
        "Wm3T": np.ascontiguousarray(np.asarray(inputs["Wm3"], f32).T).astype(BF16),
        "Wd1T": np.ascontiguousarray(np.asarray(inputs["Wd1"], f32).T).astype(BF16),
        "Wd2T": np.ascontiguousarray(np.asarray(inputs["Wd2"], f32).T).astype(BF16),
        "Wd3T": np.ascontiguousarray(np.asarray(inputs["Wd3"], f32).T).astype(BF16),
        "bhnn_r": np.stack([b_hh[2 * H:],
                            np.zeros(H, f32)])[None].astype(FP8),
        "bm1_r": np.stack([np.asarray(inputs["bm1"], f32),
                           np.zeros(H, f32)])[None].astype(FP8),
        "ident": np.eye(128, dtype=f32).astype(BF16),
        "bm2": np.asarray(inputs["bm2"], f32),
        "bm3b4": np.ascontiguousarray(np.broadcast_to(
            np.tile(np.asarray(inputs["bm3"], f32), 4), (128, 4 * A))).copy(),
        "bd1": np.asarray(inputs["bd1"], f32),
        "bd2": np.asarray(inputs["bd2"], f32),
        "bd3": np.asarray(inputs["bd3"], f32),
    }
    bos = _make_bos()
    latent = np.asarray(inputs["latent"], f32)
    target = np.asarray(inputs["target"], f32)
    in_maps = []
    for c in range(n_cores):
        sl = slice(c * bl, (c + 1) * bl)
        xT = np.zeros((T, 66, bl), np.float32)
        xT[0, :A] = bos[sl].T
        if T > 1:
            xT[1:, :A] = target[sl, 1:T].transpose(1, 2, 0)
        xT[:, A] = 1.0
        m = dict(shared)
        m["latentT"] = np.ascontiguousarray(latent[sl].T).astype(BF16)
        m["xT"] = np.ascontiguousarray(
            xT.reshape(T, 33, 2, bl)).astype(FP8)
        in_maps.append(m)
    return in_maps


_NC_CACHE = {}


def _get_nc(steps=T, reps=1):
    key = (steps, reps)
    if key not in _NC_CACHE:
        _NC_CACHE[key] = _build(steps=steps, reps=reps)
    return _NC_CACHE[key]


def kernel(**inputs):
    nc = _get_nc()
    in_maps = _make_in_maps(inputs)
    res = bass_utils.run_bass_kernel_spmd(nc, in_maps,
                                          core_ids=list(range(N_CORES)))
    bl = B // N_CORES
    y = np.empty((B, L, A), np.float32)
    for c in range(N_CORES):
        y[c * bl:(c + 1) * bl] = res.results[c]["y"]
    y[:, 0, :] = _make_bos()
    return y


# revision 52
# speedup vs baseline: 1.6041x; 1.6041x over previous
"""Trainium2 Bass kernel for nn_Decoder (latent MLP -> GRU scan -> per-step MLP).

Strategy: pure data-parallel over batch (4096 -> 8 x 512), weights replicated.
On-chip layout is fully transposed (feature dim on partitions, batch on free
dim). The recurrent matmuls (W_hh, Wm1, Wm2) run as fp8e4m3 DoubleRow
matmuls (two 128-row k-tiles per instruction at 0.5 cycles/row) with the
hidden state held in fp8; gi (x @ W_ih, K=64) stays bf16. Gate biases are
per-partition ACT bias operands and gate pre-activations accumulate in PSUM.
The pred MLP for step t-1 is emitted AFTER the gates of step t so its ACT
tanhs sit behind the chain-critical r/n/z sigmoids in ACT program order
while its matmuls fill the PE tail; x DMAs are prefetched one step ahead.
The final per-step matmul is computed batch-major so predictions land in
[B, A] orientation without transposes.

Self-contained: hardcodes shapes from the problem spec.
"""
import sys
sys.path.insert(0, "/opt/trn_rl_repo")
from contextlib import ExitStack

import numpy as np
import ml_dtypes

import concourse.bacc as bacc
import concourse.mybir as mybir
from concourse import tile
from concourse import bass_utils

BF16 = ml_dtypes.bfloat16
FP8 = ml_dtypes.float8_e4m3
BF = mybir.dt.bfloat16
F8 = mybir.dt.float8e4
F32 = mybir.dt.float32
AF = mybir.ActivationFunctionType
ALU = mybir.AluOpType
DR = mybir.MatmulPerfMode.DoubleRow

N_CORES = 8
B, LAT, H, A, L = 4096, 256, 512, 64, 128
BOS = 0
T = L - 1          # recurrence steps
BL = B // N_CORES  # per-core batch
KH = H // 128


def _build(steps=T, n_cores=N_CORES, reps=1, timing_iters=None, unroll=8,
           variant="full"):
    """Always declares the full-size DRAM interface (xT[T], y[:, L]); `steps`
    bounds the recurrence so short builds are wall-clock comparable.

    timing_iters: if set, wraps `unroll` statically-addressed step bodies in a
    hardware For_i loop executed timing_iters//unroll times (numerics garbage,
    per-step work identical) — used only to measure per-step device time."""
    nc = bacc.Bacc("TRN2", target_bir_lowering=False, debug=False,
                   num_devices=n_cores)

    d = {}
    def din(name, shape, dt=BF):
        d[name] = nc.dram_tensor(name, list(shape), dt, kind="ExternalInput").ap()

    din("latentT", [LAT, BL])
    din("xT", [T, A, BL])
    din("WhhD", [2, 128, 2, 3 * H], F8)   # k-pair-packed W_hh.T
    din("Wm1D", [2, 128, 2, H], F8)
    din("Wm2D", [2, 128, 2, A], F8)
    din("WihT", [A, 3 * H])
    din("Wm3T", [A, A])
    din("Wd1T", [LAT, H])
    din("Wd2T", [H, H])
    din("Wd3T", [H, H])
    din("b_rz", [2 * H], F32)
    din("b_inn", [H], F32)
    din("b_hnn", [H], F32)
    din("bm1", [H], F32)
    din("bm2", [A], F32)
    din("bm3b", [128, A], F32)
    din("bd1", [H], F32)
    din("bd2", [H], F32)
    din("bd3", [H], F32)
    din("ident", [128, 128])              # bf16 identity for PE adds
    y = nc.dram_tensor("y", [BL, L, A], F32, kind="ExternalOutput").ap()

    with tile.TileContext(nc) as tc, ExitStack() as ctx:
        cst = ctx.enter_context(tc.tile_pool(name="const", bufs=1))
        wrk = ctx.enter_context(tc.tile_pool(name="work", bufs=4))
        hpool = ctx.enter_context(tc.tile_pool(name="hp", bufs=4))
        ps = ctx.enter_context(tc.tile_pool(name="ps", bufs=7, space="PSUM"))
        psy = ctx.enter_context(tc.tile_pool(name="psy", bufs=1, space="PSUM"))

        def const_tile(shape, dt, tag, src):
            t = cst.tile(list(shape), dt, tag=tag, name=tag)
            nc.sync.dma_start(t[:], src)
            return t

        whhd = [const_tile([128, 2, 3 * H], F8, f"whhd{p}", d["WhhD"][p])
                for p in range(2)]
        wm1d = [const_tile([128, 2, H], F8, f"wm1d{p}", d["Wm1D"][p])
                for p in range(2)]
        wm2d = [const_tile([128, 2, A], F8, f"wm2d{p}", d["Wm2D"][p])
                for p in range(2)]
        wih = const_tile([A, 3 * H], BF, "wih", d["WihT"][:])
        wm3 = const_tile([A, A], BF, "wm3", d["Wm3T"][:])
        wd1 = [const_tile([128, H], BF, f"wd1{k}",
                          d["Wd1T"][k * 128:(k + 1) * 128, :]) for k in range(2)]
        wd2 = [const_tile([128, H], BF, f"wd2{k}",
                          d["Wd2T"][k * 128:(k + 1) * 128, :]) for k in range(KH)]
        wd3 = [const_tile([128, H], BF, f"wd3{k}",
                          d["Wd3T"][k * 128:(k + 1) * 128, :]) for k in range(KH)]

        def bias_tiles(name, n, tag):
            return [const_tile([128, 1], F32, f"{tag}{j}",
                               d[name][j * 128:(j + 1) * 128, None])
                    for j in range(n)]

        brz = bias_tiles("b_rz", 8, "brz")
        binn = bias_tiles("b_inn", KH, "binn")
        bhnn = bias_tiles("b_hnn", KH, "bhnn")
        bm1 = bias_tiles("bm1", KH, "bm1")
        bm2 = const_tile([A, 1], F32, "bm2", d["bm2"][:, None])
        bm3b = const_tile([128, A], F32, "bm3b", d["bm3b"][:])
        bd1 = bias_tiles("bd1", KH, "bd1")
        bd2 = bias_tiles("bd2", KH, "bd2")
        bd3 = bias_tiles("bd3", KH, "bd3")
        ident = const_tile([128, 128], BF, "ident", d["ident"][:])

        lat = [const_tile([128, BL], BF, f"lat{k}",
                          d["latentT"][k * 128:(k + 1) * 128, :]) for k in range(2)]

        def mlp_layer(w_tiles, rhs_tiles, bias, act, out_tag):
            outs = []
            for m in range(KH):
                acc = ps.tile([128, BL], F32, tag="ps", name="ps")
                nk = len(rhs_tiles)
                for k in range(nk):
                    nc.tensor.matmul(
                        acc[:], w_tiles[k][:, m * 128:(m + 1) * 128],
                        rhs_tiles[k][:], start=(k == 0), stop=(k == nk - 1))
                o = hpool.tile([128, BL], BF, tag=f"{out_tag}{m}",
                               name=f"{out_tag}{m}")
                nc.scalar.activation(o[:], acc[:], act, bias=bias[m][:])
                outs.append(o)
            return outs

        h1 = mlp_layer(wd1, lat, bd1, AF.Tanh, "h1")
        h2 = mlp_layer(wd2, h1, bd2, AF.Tanh, "h2")
        hb = mlp_layer(wd3, h2, bd3, AF.Identity, "hb")
        # initial fp8 hidden-state pairs: hp8[P][:, i, :] = h tile 2P+i
        hp8_init = []
        for p in range(2):
            t8 = hpool.tile([128, 2, BL], F8, tag=f"hp8{p}", name=f"hp8{p}")
            for i in range(2):
                nc.scalar.activation(t8[:, i, :], hb[2 * p + i][:], AF.Copy)
            hp8_init.append(t8)

        # hist[t] = fp8 h pair tiles of step t (init state = hist[-1]); pred
        # for step t is emitted after gates of step t+1 (see module docstring)
        state = {"hist": {-1: hp8_init}, "ystage": None, "xts": {}}
        # variant flags (dev-only timing decomposition; graded path = "full")
        want_gates = variant in ("full", "nopred")
        want_pred = variant in ("full", "mmpred")
        want_mm = variant != "eltonly"

        def fetch_x(t):
            if t in state["xts"]:
                return
            xt = wrk.tile([A, BL], BF, tag="xt", name="xt")
            nc.sync.dma_start(xt[:], d["xT"][t])
            state["xts"][t] = xt

        def gi_mm(acc, m, xt, stop):
            nc.tensor.matmul(acc[:], wih[:, m * 128:(m + 1) * 128],
                             xt[:], start=True, stop=stop)

        def gh_dr(acc, m, hp8, start, stop):
            # W_hh.T[:, m-tile] @ h as two fp8 DoubleRow matmuls (k-pairs)
            for p in range(2):
                nc.tensor.matmul(
                    acc[:], whhd[p][:, :, m * 128:(m + 1) * 128],
                    hp8[p][:, :, :],
                    start=(start and p == 0), stop=(stop and p == 1),
                    perf_mode=DR)

        def emit_gates(t, t_next):
            hp8 = state["hist"][t - 1]
            fetch_x(t)
            xt = state["xts"].pop(t)
            if t_next is not None:
                fetch_x(t_next)   # prefetch next step's x under this step

            if not want_mm:
                state["hist"][t] = hp8
                return

            # n-gate gh part first: starts the long DVE/ACT chain earliest
            hn_ps = []
            for j in range(KH):
                hn = ps.tile([128, BL], F32, tag="ps", name="ps")
                gh_dr(hn, 8 + j, hp8, start=True, stop=True)
                hn_ps.append(hn)

            r = []
            for m0 in (0, 2):
                accs = []
                for m in (m0, m0 + 1):
                    acc = ps.tile([128, BL], F32, tag="ps", name="ps")
                    gi_mm(acc, m, xt, stop=False)
                    accs.append(acc)
                for i, m in enumerate((m0, m0 + 1)):
                    gh_dr(accs[i], m, hp8, start=False, stop=True)
                if want_gates:
                    for i, m in enumerate((m0, m0 + 1)):
                        g = wrk.tile([128, BL], BF, tag=f"rz{m}",
                                     name=f"rz{m}")
                        nc.scalar.activation(g[:], accs[i][:], AF.Sigmoid,
                                             bias=brz[m][:])
                        r.append(g)

            # new fp8 h pairs for this step (halves written below)
            if want_gates:
                hp8_new = [hpool.tile([128, 2, BL], F8, tag=f"hp8{p}",
                                      name=f"hp8{p}") for p in range(2)]

            # n-gate chain: rhn -> (+inn via PE identity matmul) -> tanh -> d
            # (z matmuls run under this). The s = inn + b_inn + rhn DVE op
            # is replaced by accumulating rhn into inn's open PSUM group on
            # the PE; b_inn rides the tanh's per-partition ACT bias.
            n_list, d_list = [], []
            for j0 in (0, 2):
                inns = []
                for j in (j0, j0 + 1):
                    inn = ps.tile([128, BL], F32, tag="ps", name="ps")
                    gi_mm(inn, 8 + j, xt, stop=not want_gates)
                    inns.append(inn)
                if not want_gates:
                    continue
                for i, j in enumerate((j0, j0 + 1)):
                    rhn = wrk.tile([128, BL], BF, tag="rhn", name="rhn")
                    nc.vector.scalar_tensor_tensor(
                        rhn[:], hn_ps[j][:], bhnn[j][:], r[j][:],
                        op0=ALU.add, op1=ALU.mult)
                    nc.tensor.matmul(inns[i][:], ident[:], rhn[:],
                                     start=False, stop=True)
                    n_t = wrk.tile([128, BL], BF, tag="nt", name="nt")
                    nc.scalar.activation(n_t[:], inns[i][:], AF.Tanh,
                                         bias=binn[j][:])
                    d_t = wrk.tile([128, BL], BF, tag="dt", name="dt")
                    nc.gpsimd.tensor_sub(d_t[:], hp8[j // 2][:, j % 2, :],
                                         n_t[:])
                    n_list.append(n_t)
                    d_list.append(d_t)

            # z gate last: shortest tail (sigmoid -> zd -> h_new)
            for j0 in (0, 2):
                accs = []
                for j in (j0, j0 + 1):
                    acc = ps.tile([128, BL], F32, tag="ps", name="ps")
                    gi_mm(acc, 4 + j, xt, stop=False)
                    accs.append(acc)
                for i, j in enumerate((j0, j0 + 1)):
                    gh_dr(accs[i], 4 + j, hp8, start=False, stop=True)
                if not want_gates:
                    continue
                for i, j in enumerate((j0, j0 + 1)):
                    z = wrk.tile([128, BL], BF, tag=f"rz{4+j}",
                                 name=f"rz{4+j}")
                    nc.scalar.activation(z[:], accs[i][:], AF.Sigmoid,
                                         bias=brz[4 + j][:])
                    zd = wrk.tile([128, BL], BF, tag="zd", name="zd")
                    nc.vector.tensor_mul(zd[:], z[:], d_list[j][:])
                    # h_new = n + z*(h - n), straight to the fp8 state half
                    nc.vector.tensor_add(hp8_new[j // 2][:, j % 2, :],
                                         n_list[j][:], zd[:])
            state["hist"][t] = hp8_new if want_gates else hp8

        def emit_pred(tp, last, force=False):
            if not want_pred or (tp < 0 and not force):
                return
            hp8 = (state["hist"][tp] if tp in state["hist"]
                   else state["hist"][-1])
            ystage = state["ystage"]
            p1p8 = [wrk.tile([128, 2, BL], F8, tag=f"p1p{p}", name=f"p1p{p}")
                    for p in range(2)]
            for m in range(KH):
                acc = ps.tile([128, BL], F32, tag="ps", name="ps")
                if want_mm:
                    for p in range(2):
                        nc.tensor.matmul(
                            acc[:], wm1d[p][:, :, m * 128:(m + 1) * 128],
                            hp8[p][:, :, :],
                            start=(p == 0), stop=(p == 1), perf_mode=DR)
                nc.scalar.activation(p1p8[m // 2][:, m % 2, :], acc[:],
                                     AF.Tanh, bias=bm1[m][:])
            acc2 = ps.tile([A, BL], F32, tag="ps", name="ps")
            if want_mm:
                for p in range(2):
                    nc.tensor.matmul(acc2[:], wm2d[p][:, :, :],
                                     p1p8[p][:, :, :],
                                     start=(p == 0), stop=(p == 1),
                                     perf_mode=DR)
            p2 = wrk.tile([A, BL], BF, tag="p2", name="p2")
            nc.scalar.activation(p2[:], acc2[:], AF.Tanh, bias=bm2[:])

            tps = tp if tp >= 0 else tp + 8  # timing-build pseudo-slot
            o = (tps + 1) % 8
            g = (tps + 1) // 8
            if ystage is None or o == 0 or (g == 0 and o == 1):
                ystage = [wrk.tile([128, 8 * A], F32, tag=f"yst{bt}",
                                   name=f"yst{bt}") for bt in range(4)]
            yp = psy.tile([128, 4 * A], F32, tag="psy", name="psy")
            for bt in range(4):
                nc.tensor.matmul(yp[:, bt * A:(bt + 1) * A],
                                 p2[:, bt * 128:(bt + 1) * 128],
                                 wm3[:], start=True, stop=True)
                nc.vector.tensor_add(
                    ystage[bt][:, o * A:(o + 1) * A],
                    yp[:, bt * A:(bt + 1) * A], bm3b[:])
            if o == 7 or last:
                lo = 1 if g == 0 else 0
                hi = o + 1
                for bt in range(4):
                    nc.sync.dma_start(
                        y[bt * 128:(bt + 1) * 128, g * 8 + lo:g * 8 + hi, :],
                        ystage[bt][:, lo * A:hi * A])
            state["ystage"] = ystage
            state["hist"].pop(tp - 1, None)

        PRED_LAG = 1
        if timing_iters is None:
            for _rep in range(reps):
                for t in range(steps):
                    emit_gates(t, t + 1 if t + 1 < steps else None)
                    emit_pred(t - PRED_LAG, last=False)
                for tp in range(max(steps - PRED_LAG, 0), steps):
                    emit_pred(tp, last=(tp == steps - 1))
        else:
            # timing loop: same per-step work (preds for t<LAG read init h —
            # numerics are garbage in timing builds anyway)
            with tc.For_i(0, timing_iters // unroll, 1):
                for t in range(unroll):
                    emit_gates(t, (t + 1) % min(unroll, steps))
                    emit_pred(t - PRED_LAG, last=False, force=True)

    nc.compile()
    return nc


def _make_bos():
    bos = np.full((B, A), -16.0, np.float32)
    bos[:, BOS] = 16.0
    return bos


def _packd(WT):
    """[K, M] k-major -> [K//256, 128, 2, M] fp8 DoubleRow pair layout."""
    K, M = WT.shape
    return np.ascontiguousarray(
        WT.reshape(K // 256, 2, 128, M).transpose(0, 2, 1, 3)).astype(FP8)


def _make_in_maps(inputs, n_cores=N_CORES, T=T):
    bl = B // n_cores
    f32 = np.float32
    WhhT = np.ascontiguousarray(np.asarray(inputs["W_hh"], f32).T)
    Wm1T = np.ascontiguousarray(np.asarray(inputs["Wm1"], f32).T)
    Wm2T = np.ascontiguousarray(np.asarray(inputs["Wm2"], f32).T)
    shared = {
        "WhhD": _packd(WhhT),
        "Wm1D": _packd(Wm1T),
        "Wm2D": _packd(Wm2T),
        "WihT": np.ascontiguousarray(np.asarray(inputs["W_ih"], f32).T).astype(BF16),
        "Wm3T": np.ascontiguousarray(np.asarray(inputs["Wm3"], f32).T).astype(BF16),
        "Wd1T": np.ascontiguousarray(np.asarray(inputs["Wd1"], f32).T).astype(BF16),
        "Wd2T": np.ascontiguousarray(np.asarray(inputs["Wd2"], f32).T).astype(BF16),
        "Wd3T": np.ascontiguousarray(np.asarray(inputs["Wd3"], f32).T).astype(BF16),
        "b_rz": (np.asarray(inputs["b_ih"], f32)
                 + np.asarray(inputs["b_hh"], f32))[:2 * H].astype(f32),
        "b_inn": np.asarray(inputs["b_ih"], f32)[2 * H:].astype(f32),
        "b_hnn": np.asarray(inputs["b_hh"], f32)[2 * H:].astype(f32),
        "bm1": np.asarray(inputs["bm1"], f32),
        "bm2": np.asarray(inputs["bm2"], f32),
        "bm3b": np.ascontiguousarray(
            np.broadcast_to(np.asarray(inputs["bm3"], f32), (128, A))),
        "bd1": np.asarray(inputs["bd1"], f32),
        "bd2": np.asarray(inputs["bd2"], f32),
        "bd3": np.asarray(inputs["bd3"], f32),
        "ident": np.eye(128, dtype=f32).astype(BF16),
    }
    bos = _make_bos()
    latent = np.asarray(inputs["latent"], f32)
    target = np.asarray(inputs["target"], f32)
    in_maps = []
    for c in range(n_cores):
        sl = slice(c * bl, (c + 1) * bl)
        xT = np.empty((T, A, bl), BF16)
        xT[0] = bos[sl].T
        if T > 1:
            xT[1:] = target[sl, 1:T].transpose(1, 2, 0).astype(BF16)
        m = dict(shared)
        m["latentT"] = np.ascontiguousarray(latent[sl].T).astype(BF16)
        m["xT"] = xT
        in_maps.append(m)
    return in_maps


_NC_CACHE = {}


def _get_nc(steps=T, reps=1):
    key = (steps, reps)
    if key not in _NC_CACHE:
        _NC_CACHE[key] = _build(steps=steps, reps=reps)
    return _NC_CACHE[key]


def kernel(**inputs):
    nc = _get_nc()
    in_maps = _make_in_maps(inputs)
    res = bass_utils.run_bass_kernel_spmd(nc, in_maps,
                                          core_ids=list(range(N_CORES)))
    bl = B // N_CORES
    y = np.empty((B, L, A), np.float32)
    for c in range(N_CORES):
        y[c * bl:(c + 1) * bl] = res.results[c]["y"]
    y[:, 0, :] = _make_bos()
    return y



# revision 68
# speedup vs baseline: 1.8262x; 1.1385x over previous
"""Trainium2 Bass kernel for nn_Decoder (latent MLP -> GRU scan -> per-step MLP).

Strategy: pure data-parallel over batch (4096 -> 8 x 512), weights replicated.
On-chip layout is fully transposed (feature dim on partitions, batch on free
dim). The recurrent matmuls (W_hh, Wm1, Wm2) run as fp8e4m3 DoubleRow
matmuls (two 128-row k-tiles per instruction at 0.5 cycles/row) with the
hidden state held in fp8; gi (x @ W_ih, K=64) stays bf16. Gate biases are
per-partition ACT bias operands and gate pre-activations accumulate in PSUM.
The pred MLP for step t-1 is emitted AFTER the gates of step t so its ACT
tanhs sit behind the chain-critical r/n/z sigmoids in ACT program order
while its matmuls fill the PE tail; x DMAs are prefetched one step ahead.
The final per-step matmul is computed batch-major so predictions land in
[B, A] orientation without transposes.

Self-contained: hardcodes shapes from the problem spec.
"""
import sys
sys.path.insert(0, "/opt/trn_rl_repo")
from contextlib import ExitStack

import numpy as np
import ml_dtypes

import concourse.bacc as bacc
import concourse.mybir as mybir
from concourse import tile
from concourse import bass_utils

BF16 = ml_dtypes.bfloat16
FP8 = ml_dtypes.float8_e4m3
BF = mybir.dt.bfloat16
F8 = mybir.dt.float8e4
F32 = mybir.dt.float32
AF = mybir.ActivationFunctionType
ALU = mybir.AluOpType
DR = mybir.MatmulPerfMode.DoubleRow

N_CORES = 8
B, LAT, H, A, L = 4096, 256, 512, 64, 128
BOS = 0
T = L - 1          # recurrence steps
BL = B // N_CORES  # per-core batch
KH = H // 128


def _build(steps=T, n_cores=N_CORES, reps=1, timing_iters=None, unroll=8,
           variant="full"):
    """Always declares the full-size DRAM interface (xT[T], y[:, L]); `steps`
    bounds the recurrence so short builds are wall-clock comparable.

    timing_iters: if set, wraps `unroll` statically-addressed step bodies in a
    hardware For_i loop executed timing_iters//unroll times (numerics garbage,
    per-step work identical) — used only to measure per-step device time."""
    nc = bacc.Bacc("TRN2", target_bir_lowering=False, debug=False,
                   num_devices=n_cores)

    d = {}
    def din(name, shape, dt=BF):
        d[name] = nc.dram_tensor(name, list(shape), dt, kind="ExternalInput").ap()

    din("latentT", [LAT, BL])
    din("xT", [T, A, BL])
    din("WhhD", [2, 128, 2, 3 * H], F8)   # k-pair-packed W_hh.T
    din("Wm1D", [2, 128, 2, H], F8)
    din("Wm2D", [2, 128, 2, A], F8)
    din("WihT", [A, 3 * H])
    din("Wm3T", [A, A])
    din("Wd1T", [LAT, H])
    din("Wd2T", [H, H])
    din("Wd3T", [H, H])
    din("b_rz", [2 * H], F32)
    din("b_inn", [H], F32)
    din("b_hnn", [H], F32)
    din("bm1", [H], F32)
    din("bm2", [A], F32)
    din("bm3b", [128, A], F32)
    din("bd1", [H], F32)
    din("bd2", [H], F32)
    din("bd3", [H], F32)
    din("ident", [128, 128])              # bf16 identity for PE adds
    y = nc.dram_tensor("y", [BL, L, A], F32, kind="ExternalOutput").ap()

    with tile.TileContext(nc) as tc, ExitStack() as ctx:
        cst = ctx.enter_context(tc.tile_pool(name="const", bufs=1))
        wrk = ctx.enter_context(tc.tile_pool(name="work", bufs=4))
        hpool = ctx.enter_context(tc.tile_pool(name="hp", bufs=4))
        ps = ctx.enter_context(tc.tile_pool(name="ps", bufs=7, space="PSUM"))
        psy = ctx.enter_context(tc.tile_pool(name="psy", bufs=1, space="PSUM"))

        def const_tile(shape, dt, tag, src):
            t = cst.tile(list(shape), dt, tag=tag, name=tag)
            nc.sync.dma_start(t[:], src)
            return t

        whhd = [const_tile([128, 2, 3 * H], F8, f"whhd{p}", d["WhhD"][p])
                for p in range(2)]
        wm1d = [const_tile([128, 2, H], F8, f"wm1d{p}", d["Wm1D"][p])
                for p in range(2)]
        wm2d = [const_tile([128, 2, A], F8, f"wm2d{p}", d["Wm2D"][p])
                for p in range(2)]
        wih = const_tile([A, 3 * H], BF, "wih", d["WihT"][:])
        wm3 = const_tile([A, A], BF, "wm3", d["Wm3T"][:])
        wd1 = [const_tile([128, H], BF, f"wd1{k}",
                          d["Wd1T"][k * 128:(k + 1) * 128, :]) for k in range(2)]
        wd2 = [const_tile([128, H], BF, f"wd2{k}",
                          d["Wd2T"][k * 128:(k + 1) * 128, :]) for k in range(KH)]
        wd3 = [const_tile([128, H], BF, f"wd3{k}",
                          d["Wd3T"][k * 128:(k + 1) * 128, :]) for k in range(KH)]

        def bias_tiles(name, n, tag):
            return [const_tile([128, 1], F32, f"{tag}{j}",
                               d[name][j * 128:(j + 1) * 128, None])
                    for j in range(n)]

        brz = bias_tiles("b_rz", 8, "brz")
        binn = bias_tiles("b_inn", KH, "binn")
        bhnn = bias_tiles("b_hnn", KH, "bhnn")
        bm1 = bias_tiles("bm1", KH, "bm1")
        bm2 = const_tile([A, 1], F32, "bm2", d["bm2"][:, None])
        bm3b = const_tile([128, A], F32, "bm3b", d["bm3b"][:])
        bd1 = bias_tiles("bd1", KH, "bd1")
        bd2 = bias_tiles("bd2", KH, "bd2")
        bd3 = bias_tiles("bd3", KH, "bd3")
        ident = const_tile([128, 128], BF, "ident", d["ident"][:])

        lat = [const_tile([128, BL], BF, f"lat{k}",
                          d["latentT"][k * 128:(k + 1) * 128, :]) for k in range(2)]

        def mlp_layer(w_tiles, rhs_tiles, bias, act, out_tag):
            outs = []
            for m in range(KH):
                acc = ps.tile([128, BL], F32, tag="ps", name="ps")
                nk = len(rhs_tiles)
                for k in range(nk):
                    nc.tensor.matmul(
                        acc[:], w_tiles[k][:, m * 128:(m + 1) * 128],
                        rhs_tiles[k][:], start=(k == 0), stop=(k == nk - 1))
                o = hpool.tile([128, BL], BF, tag=f"{out_tag}{m}",
                               name=f"{out_tag}{m}")
                nc.scalar.activation(o[:], acc[:], act, bias=bias[m][:])
                outs.append(o)
            return outs

        h1 = mlp_layer(wd1, lat, bd1, AF.Tanh, "h1")
        h2 = mlp_layer(wd2, h1, bd2, AF.Tanh, "h2")
        hb = mlp_layer(wd3, h2, bd3, AF.Identity, "hb")
        # initial fp8 hidden-state pairs: hp8[P][:, i, :] = h tile 2P+i
        hp8_init = []
        for p in range(2):
            t8 = hpool.tile([128, 2, BL], F8, tag=f"hp8{p}", name=f"hp8{p}")
            for i in range(2):
                nc.scalar.activation(t8[:, i, :], hb[2 * p + i][:], AF.Copy)
            hp8_init.append(t8)

        # hist[t] = fp8 h pair tiles of step t (init state = hist[-1]); pred
        # for step t is emitted after gates of step t+1 (see module docstring)
        state = {"hist": {-1: hp8_init}, "ystage": None, "xts": {}}
        # variant flags (dev-only timing decomposition; graded path = "full")
        want_gates = variant in ("full", "nopred")
        want_pred = variant in ("full", "mmpred")
        want_mm = variant != "eltonly"

        def fetch_x(t):
            if t in state["xts"]:
                return
            xt = wrk.tile([A, BL], BF, tag="xt", name="xt")
            nc.sync.dma_start(xt[:], d["xT"][t])
            state["xts"][t] = xt

        def gi_mm(acc, m, xt, stop):
            nc.tensor.matmul(acc[:], wih[:, m * 128:(m + 1) * 128],
                             xt[:], start=True, stop=stop)

        def gh_dr(acc, m, hp8, start, stop):
            # W_hh.T[:, m-tile] @ h as two fp8 DoubleRow matmuls (k-pairs)
            for p in range(2):
                nc.tensor.matmul(
                    acc[:], whhd[p][:, :, m * 128:(m + 1) * 128],
                    hp8[p][:, :, :],
                    start=(start and p == 0), stop=(stop and p == 1),
                    perf_mode=DR)

        def emit_gates(t, t_next):
            hp8 = state["hist"][t - 1]
            fetch_x(t)
            xt = state["xts"].pop(t)
            if t_next is not None:
                fetch_x(t_next)   # prefetch next step's x under this step

            if not want_mm:
                state["hist"][t] = hp8
                return

            # n-gate gh part first: starts the long DVE/ACT chain earliest
            hn_ps = []
            with tc.high_priority():
                for j in range(KH):
                    hn = ps.tile([128, BL], F32, tag="ps", name="ps")
                    gh_dr(hn, 8 + j, hp8, start=True, stop=True)
                    hn_ps.append(hn)

            r = []
            for m0 in (0, 2):
                accs = []
                for m in (m0, m0 + 1):
                    acc = ps.tile([128, BL], F32, tag="ps", name="ps")
                    gi_mm(acc, m, xt, stop=False)
                    accs.append(acc)
                for i, m in enumerate((m0, m0 + 1)):
                    gh_dr(accs[i], m, hp8, start=False, stop=True)
                if want_gates:
                    for i, m in enumerate((m0, m0 + 1)):
                        g = wrk.tile([128, BL], BF, tag=f"rz{m}",
                                     name=f"rz{m}")
                        nc.scalar.activation(g[:], accs[i][:], AF.Sigmoid,
                                             bias=brz[m][:])
                        r.append(g)

            # new fp8 h pairs for this step (halves written below)
            if want_gates:
                hp8_new = [hpool.tile([128, 2, BL], F8, tag=f"hp8{p}",
                                      name=f"hp8{p}") for p in range(2)]

            # n-gate chain: rhn -> (+inn via PE identity matmul) -> tanh -> d
            # (z matmuls run under this). The s = inn + b_inn + rhn DVE op
            # is replaced by accumulating rhn into inn's open PSUM group on
            # the PE; b_inn rides the tanh's per-partition ACT bias.
            n_list, d_list = [], []
            for j0 in (0, 2):
                inns = []
                for j in (j0, j0 + 1):
                    inn = ps.tile([128, BL], F32, tag="ps", name="ps")
                    gi_mm(inn, 8 + j, xt, stop=not want_gates)
                    inns.append(inn)
                if not want_gates:
                    continue
                for i, j in enumerate((j0, j0 + 1)):
                    with tc.high_priority():
                        rhn = wrk.tile([128, BL], BF, tag="rhn",
                                       name="rhn")
                        nc.vector.scalar_tensor_tensor(
                            rhn[:], hn_ps[j][:], bhnn[j][:], r[j][:],
                            op0=ALU.add, op1=ALU.mult)
                        nc.tensor.matmul(inns[i][:], ident[:], rhn[:],
                                         start=False, stop=True)
                        n_t = wrk.tile([128, BL], BF, tag="nt", name="nt")
                        nc.scalar.activation(n_t[:], inns[i][:], AF.Tanh,
                                             bias=binn[j][:])
                    d_t = wrk.tile([128, BL], BF, tag="dt", name="dt")
                    nc.gpsimd.tensor_sub(d_t[:], hp8[j // 2][:, j % 2, :],
                                         n_t[:])
                    n_list.append(n_t)
                    d_list.append(d_t)

            # z gate last: shortest tail (sigmoid -> zd -> h_new)
            for j0 in (0, 2):
                accs = []
                for j in (j0, j0 + 1):
                    acc = ps.tile([128, BL], F32, tag="ps", name="ps")
                    gi_mm(acc, 4 + j, xt, stop=False)
                    accs.append(acc)
                for i, j in enumerate((j0, j0 + 1)):
                    gh_dr(accs[i], 4 + j, hp8, start=False, stop=True)
                if not want_gates:
                    continue
                for i, j in enumerate((j0, j0 + 1)):
                    z = wrk.tile([128, BL], BF, tag=f"rz{4+j}",
                                 name=f"rz{4+j}")
                    nc.scalar.activation(z[:], accs[i][:], AF.Sigmoid,
                                         bias=brz[4 + j][:])
                    zd = wrk.tile([128, BL], BF, tag="zd", name="zd")
                    nc.vector.tensor_mul(zd[:], z[:], d_list[j][:])
                    # h_new = n + z*(h - n), straight to the fp8 state half
                    nc.vector.tensor_add(hp8_new[j // 2][:, j % 2, :],
                                         n_list[j][:], zd[:])
            state["hist"][t] = hp8_new if want_gates else hp8

        def emit_pred(tp, last, force=False):
            if not want_pred or (tp < 0 and not force):
                return
            hp8 = (state["hist"][tp] if tp in state["hist"]
                   else state["hist"][-1])
            ystage = state["ystage"]
            p1p8 = [wrk.tile([128, 2, BL], F8, tag=f"p1p{p}", name=f"p1p{p}")
                    for p in range(2)]
            for m in range(KH):
                acc = ps.tile([128, BL], F32, tag="ps", name="ps")
                if want_mm:
                    for p in range(2):
                        nc.tensor.matmul(
                            acc[:], wm1d[p][:, :, m * 128:(m + 1) * 128],
                            hp8[p][:, :, :],
                            start=(p == 0), stop=(p == 1), perf_mode=DR)
                nc.scalar.activation(p1p8[m // 2][:, m % 2, :], acc[:],
                                     AF.Tanh, bias=bm1[m][:])
            acc2 = ps.tile([A, BL], F32, tag="ps", name="ps")
            if want_mm:
                for p in range(2):
                    nc.tensor.matmul(acc2[:], wm2d[p][:, :, :],
                                     p1p8[p][:, :, :],
                                     start=(p == 0), stop=(p == 1),
                                     perf_mode=DR)
            p2 = wrk.tile([A, BL], BF, tag="p2", name="p2")
            nc.scalar.activation(p2[:], acc2[:], AF.Tanh, bias=bm2[:])

            tps = tp if tp >= 0 else tp + 8  # timing-build pseudo-slot
            o = (tps + 1) % 8
            g = (tps + 1) // 8
            if ystage is None or o == 0 or (g == 0 and o == 1):
                ystage = [wrk.tile([128, 8 * A], F32, tag=f"yst{bt}",
                                   name=f"yst{bt}") for bt in range(4)]
            yp = psy.tile([128, 4 * A], F32, tag="psy", name="psy")
            for bt in range(4):
                nc.tensor.matmul(yp[:, bt * A:(bt + 1) * A],
                                 p2[:, bt * 128:(bt + 1) * 128],
                                 wm3[:], start=True, stop=True)
                nc.vector.tensor_add(
                    ystage[bt][:, o * A:(o + 1) * A],
                    yp[:, bt * A:(bt + 1) * A], bm3b[:])
            if o == 7 or last:
                lo = 1 if g == 0 else 0
                hi = o + 1
                for bt in range(4):
                    nc.sync.dma_start(
                        y[bt * 128:(bt + 1) * 128, g * 8 + lo:g * 8 + hi, :],
                        ystage[bt][:, lo * A:hi * A])
            state["ystage"] = ystage
            state["hist"].pop(tp - 1, None)

        PRED_LAG = 1
        if timing_iters is None:
            for _rep in range(reps):
                for t in range(steps):
                    emit_gates(t, t + 1 if t + 1 < steps else None)
                    emit_pred(t - PRED_LAG, last=False)
                for tp in range(max(steps - PRED_LAG, 0), steps):
                    emit_pred(tp, last=(tp == steps - 1))
        else:
            # timing loop: same per-step work (preds for t<LAG read init h —
            # numerics are garbage in timing builds anyway)
            with tc.For_i(0, timing_iters // unroll, 1):
                for t in range(unroll):
                    emit_gates(t, (t + 1) % min(unroll, steps))
                    emit_pred(t - PRED_LAG, last=False, force=True)

    nc.compile()
    return nc


def _make_bos():
    bos = np.full((B, A), -16.0, np.float32)
    bos[:, BOS] = 16.0
    return bos


def _packd(WT):
    """[K, M] k-major -> [K//256, 128, 2, M] fp8 DoubleRow pair layout."""
    K, M = WT.shape
    return np.ascontiguousarray(
        WT.reshape(K // 256, 2, 128, M).transpose(0, 2, 1, 3)).astype(FP8)


def _make_in_maps(inputs, n_cores=N_CORES, T=T):
    bl = B // n_cores
    f32 = np.float32
    WhhT = np.ascontiguousarray(np.asarray(inputs["W_hh"], f32).T)
    Wm1T = np.ascontiguousarray(np.asarray(inputs["Wm1"], f32).T)
    Wm2T = np.ascontiguousarray(np.asarray(inputs["Wm2"], f32).T)
    shared = {
        "WhhD": _packd(WhhT),
        "Wm1D": _packd(Wm1T),
        "Wm2D": _packd(Wm2T),
        "WihT": np.ascontiguousarray(np.asarray(inputs["W_ih"], f32).T).astype(BF16),
        "Wm3T": np.ascontiguousarray(np.asarray(inputs["Wm3"], f32).T).astype(BF16),
        "Wd1T": np.ascontiguousarray(np.asarray(inputs["Wd1"], f32).T).astype(BF16),
        "Wd2T": np.ascontiguousarray(np.asarray(inputs["Wd2"], f32).T).astype(BF16),
        "Wd3T": np.ascontiguousarray(np.asarray(inputs["Wd3"], f32).T).astype(BF16),
        "b_rz": (np.asarray(inputs["b_ih"], f32)
                 + np.asarray(inputs["b_hh"], f32))[:2 * H].astype(f32),
        "b_inn": np.asarray(inputs["b_ih"], f32)[2 * H:].astype(f32),
        "b_hnn": np.asarray(inputs["b_hh"], f32)[2 * H:].astype(f32),
        "bm1": np.asarray(inputs["bm1"], f32),
        "bm2": np.asarray(inputs["bm2"], f32),
        "bm3b": np.ascontiguousarray(
            np.broadcast_to(np.asarray(inputs["bm3"], f32), (128, A))),
        "bd1": np.asarray(inputs["bd1"], f32),
        "bd2": np.asarray(inputs["bd2"], f32),
        "bd3": np.asarray(inputs["bd3"], f32),
        "ident": np.eye(128, dtype=f32).astype(BF16),
    }
    bos = _make_bos()
    latent = np.asarray(inputs["latent"], f32)
    target = np.asarray(inputs["target"], f32)
    in_maps = []
    for c in range(n_cores):
        sl = slice(c * bl, (c + 1) * bl)
        xT = np.empty((T, A, bl), BF16)
        xT[0] = bos[sl].T
        if T > 1:
            xT[1:] = target[sl, 1:T].transpose(1, 2, 0).astype(BF16)
        m = dict(shared)
        m["latentT"] = np.ascontiguousarray(latent[sl].T).astype(BF16)
        m["xT"] = xT
        in_maps.append(m)
    return in_maps


_NC_CACHE = {}


def _get_nc(steps=T, reps=1):
    key = (steps, reps)
    if key not in _NC_CACHE:
        _NC_CACHE[key] = _build(steps=steps, reps=reps)
    return _NC_CACHE[key]


def kernel(**inputs):
    nc = _get_nc()
    in_maps = _make_in_maps(inputs)
    res = bass_utils.run_bass_kernel_spmd(nc, in_maps,
                                          core_ids=list(range(N_CORES)))
    bl = B // N_CORES
    y = np.empty((B, L, A), np.float32)
    for c in range(N_CORES):
        y[c * bl:(c + 1) * bl] = res.results[c]["y"]
    y[:, 0, :] = _make_bos()
    return y

